# revision 3
# baseline (speedup 1.0000x reference)
"""Causal single-head attention on 8 Trainium2 NeuronCores (Bass/Tile).

Problem: X [4, 2048, 1024] f32; W_q/W_k/W_v [1024, 1024] f32.
out[b] = softmax(mask((X[b] Wq)(X[b] Wk)^T / 32)) (X[b] Wv)

Sharding: 8 cores = 4 batches x 2 key-parity halves (partial softmax).
Core c = 2b + h owns batch b's key tiles {2j + h : j = 0..7} (128-row
tiles, interleaved so causal work per local tile j is j-independent across
cores). Each core computes unnormalized partial attention over its own
keys only and returns the partial numerator [2048, 1024] plus partial
softmax denominators; the host adds each pair's partials and divides.
Since exp needs no max-subtraction here (|scores/32| < ~4), partial
softmax combines exactly.

Merged QK weight: scores = (X Wq)(Xk Wk)^T = X (Wq Wk^T) Xk^T, so the
host folds wkq := Wk Wq^T once and the kernel computes scores as
(Xk wkq) X^T - the entire Q projection becomes a raw DMA of X^T.

fp8 DoubleRow: e4m3 matmuls in DoubleRow perf mode contract 256 rows
(two 128-partition tiles side by side in the free dim) at 0.5 PE
cycles/output-row - 4x bf16 throughput. Used for the KW projection (M1),
the V projection (M2) and the score matmul (M3). Quantization error is
controlled with same-scale hi+lo splits (lo = fp8(x - hi), leaning on
e4m3 denormals; both terms accumulate in one fp32 PSUM group):
  M1: KW = xk_hi@wm_hi + xk_lo@wm_hi + xk_hi@wm_lo  (splits on host)
  M2: 2048*V = xk_hi@wv_hi + xk_lo@wv_hi + xk_hi@wv_lo
  M3: s*64 = KW8^T @ xq_hi            (KW requantized to fp8 on copy)
wm is prescaled x64 (else its entries sit in e4m3's denormal range) and
wv x2048; the scales fold into the exp activation scale (1/(32*64)) and
the V copy scale. The softmax/V path stays >= fp16: wt = exp fp16,
V fp16 (copy scale 1/(2048*16)), numerator matmul fp16 (same PE cost as
bf16, 8 more mantissa bits), fp16 outputs (den uses ones=1/16 so
Sum(exp) fits fp16; the /16 cancels in num/den on the host).

Schedule: phase A runs all of M1 before M2 so the early DMA stream only
has to feed the merged weight + Xk tiles; the DMA issue order tracks the
per-psum matmul term order (hh, lh, hl). Phase B interleaves scores and
numerators per key tile j; the numerators for the last q-tiles (g=14,15)
pre-accumulate their jj<=6 terms into held PSUM banks before the j=7
scores so only 4 closing matmuls + copies + DMA remain after the last
exp - that cuts the end-of-kernel flush tail.
"""

import sys

if "/opt/trn_rl_repo" not in sys.path:
    sys.path.insert(0, "/opt/trn_rl_repo")

import numpy as np

B, S, D = 4, 2048, 1024
HK = S // 2  # own key rows per core
P = 128
N_CORES = 8
SW = 64.0  # wm prescale (folded into exp scale)
WS = 2048.0  # wv prescale (folded into V copy scale)
# column offset of attention-weight block j inside the packed wt tile
WOFF = [0] * 9
for _j in range(8):
    WOFF[_j + 1] = WOFF[_j] + (16 - 2 * _j) * P
WTW = WOFF[8]  # 9216

# M1/M2 split terms: (xk part, weight part); "h"/"l" = hi/lo.
M1_TERMS = (("h", "h"), ("l", "h"), ("h", "l"))
M2_TERMS = (("h", "h"), ("l", "h"), ("h", "l"))

_cache = {}


def _score_chunks(j):
    """(q0, width) chunks covering q in [256j, 2048): 512-wide + 256 rem."""
    out = []
    q0, w = 256 * j, (16 - 2 * j) * P
    while w > 0:
        cw = 512 if w >= 512 else w
        out.append((q0, cw))
        q0 += cw
        w -= cw
    return out


def _build_nc():
    from concourse import bacc
    import concourse.mybir as mybir
    import concourse.tile as tile

    fp32 = mybir.dt.float32
    fp16 = mybir.dt.float16
    fp8 = mybir.dt.float8e4
    Exp = mybir.ActivationFunctionType.Exp
    Copy = mybir.ActivationFunctionType.Copy
    DR = mybir.MatmulPerfMode.DoubleRow

    nc = bacc.Bacc("TRN2", target_bir_lowering=False)

    xkh_d = nc.dram_tensor("xkh", [D, HK], fp8, kind="ExternalInput")
    xkl_d = nc.dram_tensor("xkl", [D, HK], fp8, kind="ExternalInput")
    xq_d = nc.dram_tensor("xq", [D, S], fp8, kind="ExternalInput")
    wmh_d = nc.dram_tensor("wmh", [D, D], fp8, kind="ExternalInput")
    wml_d = nc.dram_tensor("wml", [D, D], fp8, kind="ExternalInput")
    wvh_d = nc.dram_tensor("wvh", [D, D], fp8, kind="ExternalInput")
    wvl_d = nc.dram_tensor("wvl", [D, D], fp8, kind="ExternalInput")
    band_d = nc.dram_tensor("band", [P, 256], fp16, kind="ExternalInput")
    # num columns 0:1024 = partial numerator; column 1024 = denominator/16
    num_d = nc.dram_tensor("num", [S, D + 1], fp16, kind="ExternalOutput")

    xkh3 = xkh_d.rearrange("(o p) s -> p o s", p=P)
    xkl3 = xkl_d.rearrange("(o p) s -> p o s", p=P)
    xq3 = xq_d.rearrange("(o p) q -> p o q", p=P)
    wmh3 = wmh_d.rearrange("(o p) e -> p o e", p=P)
    wml3 = wml_d.rearrange("(o p) e -> p o e", p=P)
    wvh3 = wvh_d.rearrange("(o p) e -> p o e", p=P)
    wvl3 = wvl_d.rearrange("(o p) e -> p o e", p=P)

    with tile.TileContext(nc) as tc:
        with (
            tc.tile_pool(name="persist", bufs=1) as persist,
            tc.tile_pool(name="psS", bufs=2, space="PSUM") as psS,
            tc.tile_pool(name="psAV", bufs=4, space="PSUM") as psAV,
            tc.tile_pool(name="psD", bufs=1, space="PSUM") as psD,
        ):
            XQ = persist.tile([P, 8, S], fp8, tag="xq")  # fp8 X^T, all q
            XKh = persist.tile([P, 8, HK], fp8, tag="xkh")
            XKl = persist.tile([P, 8, HK], fp8, tag="xkl")
            KW = persist.tile([P, 8, HK], fp8, tag="kw")  # (Xk wkq)^T fp8
            V = persist.tile([P, 8, D], fp16, tag="v")  # V/16
            band = persist.tile([P, 256], fp16, tag="band")
            ones = persist.tile([P, 1], fp16, tag="ones")
            nc.vector.memset(ones[:], 1.0 / 16.0)

            xk3 = {"h": XKh, "l": XKl}

            # ---- Phase A: projections (fp8 DoubleRow, fp32 PSUM) ----
            with tc.tile_pool(name="wts", bufs=1) as wp:
                WMh = wp.tile([P, 8, D], fp8, tag="wmh")
                WMl = wp.tile([P, 8, D], fp8, tag="wml")
                WVh = wp.tile([P, 8, D], fp8, tag="wvh")
                WVl = wp.tile([P, 8, D], fp8, tag="wvl")
                wm = {"h": WMh, "l": WMl}
                wv = {"h": WVh, "l": WVl}

                # DMA order tracks the matmul term order (hh, lh, hl) so the
                # first M1 psums never wait long: first the e=0 slices, then
                # progressively wider weight column chunks.
                nc.sync.dma_start(WMh[:, :, :P], wmh3[:, :, :P])
                nc.sync.dma_start(XKh[:, :, :256], xkh3[:, :, :256])
                nc.sync.dma_start(XKh[:, :, 256:512], xkh3[:, :, 256:512])
                nc.sync.dma_start(XKl[:, :, :512], xkl3[:, :, :512])
                nc.sync.dma_start(WMl[:, :, :P], wml3[:, :, :P])
                nc.sync.dma_start(WMh[:, :, P:512], wmh3[:, :, P:512])
                nc.sync.dma_start(WMl[:, :, P:512], wml3[:, :, P:512])
                nc.sync.dma_start(WMh[:, :, 512:], wmh3[:, :, 512:])
                nc.sync.dma_start(WMl[:, :, 512:], wml3[:, :, 512:])
                nc.sync.dma_start(XKh[:, :, 512:], xkh3[:, :, 512:])
                nc.sync.dma_start(XKl[:, :, 512:], xkl3[:, :, 512:])
                nc.sync.dma_start(WVh[:], wvh3[:])
                nc.sync.dma_start(WVl[:], wvl3[:])
                for qsc in range(4):
                    nc.sync.dma_start(
                        XQ[:, :, qsc * 512 : (qsc + 1) * 512],
                        xq3[:, :, qsc * 512 : (qsc + 1) * 512],
                    )
                nc.sync.dma_start(band[:], band_d[:])

                # M1: KW[e, k] = sum_d wm[d, e] * xkT[d, k] (DR d-pairs)
                for sc in range(2):
                    for e in range(8):
                        psum = psAV.tile([P, 512], fp32, tag="psAV")
                        n = 4 * len(M1_TERMS)
                        i = 0
                        # first psum in column halves: its inputs arrive in
                        # two DMA chunks, so the PE can start ~1us sooner
                        halves = (
                            ((0, 256), (256, 512)) if sc == 0 and e == 0 else ((0, 512),)
                        )
                        for xa, wb_ in M1_TERMS:
                            for lo, hi in halves:
                                for dp in range(4):
                                    nc.tensor.matmul(
                                        psum[:, lo:hi],
                                        wm[wb_][:, 2 * dp : 2 * dp + 2, e * P : (e + 1) * P],
                                        xk3[xa][:, 2 * dp : 2 * dp + 2, sc * 512 + lo : sc * 512 + hi],
                                        start=(i == 0 or i == len(halves) * 4),
                                        stop=(i >= n * len(halves) - 4),
                                        perf_mode=DR,
                                    )
                                    i += 1
                        nc.vector.tensor_copy(
                            out=KW[:, e, sc * 512 : (sc + 1) * 512], in_=psum[:]
                        )
                # M2: 2048*16*(V[k, e]/16) = sum_d xkT[d, k] * wv'[d, e]
                for sc in range(2):
                    for kti in range(4):
                        kt = 4 * sc + kti
                        for ec in range(2):
                            psum = psAV.tile([P, 512], fp32, tag="psAV")
                            n = 4 * len(M2_TERMS)
                            i = 0
                            for xa, wb_ in M2_TERMS:
                                for dp in range(4):
                                    nc.tensor.matmul(
                                        psum[:],
                                        xk3[xa][:, 2 * dp : 2 * dp + 2, kt * P : (kt + 1) * P],
                                        wv[wb_][:, 2 * dp : 2 * dp + 2, ec * 512 : (ec + 1) * 512],
                                        start=(i == 0),
                                        stop=(i == n - 1),
                                        perf_mode=DR,
                                    )
                                    i += 1
                            nc.scalar.activation(
                                V[:, kt, ec * 512 : (ec + 1) * 512],
                                psum[:],
                                Copy,
                                scale=1.0 / (WS * 16.0),
                            )

            # ---- Phase B: partial attention over own key tiles ----
            with (
                tc.tile_pool(name="wtp", bufs=1) as wtp,
                tc.tile_pool(name="outp", bufs=2) as outp,
            ):
                wt = wtp.tile([P, WTW], fp16, tag="wt")

                def scores_for(j):
                    wcol = WOFF[j]
                    for q0, cw in _score_chunks(j):
                        psum_s = psS.tile([P, 512], fp32, tag="psS")
                        for ep in range(4):
                            nc.tensor.matmul(
                                psum_s[:, :cw],
                                KW[:, 2 * ep : 2 * ep + 2, j * P : (j + 1) * P],
                                XQ[:, 2 * ep : 2 * ep + 2, q0 : q0 + cw],
                                start=(ep == 0),
                                stop=(ep == 3),
                                perf_mode=DR,
                            )
                        nc.scalar.activation(
                            wt[:, wcol : wcol + cw],
                            psum_s[:, :cw],
                            Exp,
                            scale=1.0 / (32.0 * SW),
                        )
                        if q0 == 256 * j:
                            # diagonal block: causal 0/1 mask (parity in data)
                            nc.vector.tensor_mul(
                                wt[:, wcol : wcol + 256],
                                wt[:, wcol : wcol + 256],
                                band[:],
                            )
                        wcol += cw

                def wt_blk(g, jj):
                    return wt[:, WOFF[jj] + (g - 2 * jj) * P :][:, :P]

                def emit_den(g, nj, out_sb):
                    psum_dn = psD.tile([P, 1], fp32, tag="psD")
                    for jj in range(nj):
                        nc.tensor.matmul(
                            psum_dn[:],
                            wt_blk(g, jj),
                            ones[:],
                            start=(jj == 0),
                            stop=(jj == nj - 1),
                        )
                    nc.vector.tensor_copy(out=out_sb[:, D : D + 1], in_=psum_dn[:])

                def emit_g(g):
                    nj = g // 2 + 1  # own key tiles jj with 2jj <= g
                    out_sb = outp.tile([P, D + 1], fp16, tag="out")
                    for ec in range(2):
                        psum_av = psAV.tile([P, 512], fp32, tag="psAV")
                        for jj in range(nj):
                            nc.tensor.matmul(
                                psum_av[:],
                                wt_blk(g, jj),
                                V[:, jj, ec * 512 : (ec + 1) * 512],
                                start=(jj == 0),
                                stop=(jj == nj - 1),
                            )
                        eng = nc.vector if ec == 0 else nc.gpsimd
                        eng.tensor_copy(
                            out=out_sb[:, ec * 512 : (ec + 1) * 512], in_=psum_av[:]
                        )
                        if ec == 0:
                            nc.sync.dma_start(
                                num_d[g * P : (g + 1) * P, :512], out_sb[:, :512]
                            )
                            emit_den(g, nj, out_sb)
                    nc.sync.dma_start(
                        num_d[g * P : (g + 1) * P, 512:], out_sb[:, 512:]
                    )

                for j in range(7):
                    scores_for(j)
                    for g in (2 * j, 2 * j + 1):
                        emit_g(g)

                # tail: pre-accumulate g=14,15 over jj<=6 in held PSUM banks,
                # then after the last scores only 4 closing matmuls + the
                # denominators + copies + DMA remain
                pre = {}
                for g in (14, 15):
                    for ec in range(2):
                        psum_av = psAV.tile([P, 512], fp32, tag="psAV")
                        for jj in range(7):
                            nc.tensor.matmul(
                                psum_av[:],
                                wt_blk(g, jj),
                                V[:, jj, ec * 512 : (ec + 1) * 512],
                                start=(jj == 0),
                                stop=False,
                            )
                        pre[(g, ec)] = psum_av
                scores_for(7)
                for g in (14, 15):
                    out_sb = outp.tile([P, D + 1], fp16, tag="out")
                    for ec in range(2):
                        psum_av = pre[(g, ec)]
                        nc.tensor.matmul(
                            psum_av[:],
                            wt_blk(g, 7),
                            V[:, 7, ec * 512 : (ec + 1) * 512],
                            start=False,
                            stop=True,
                        )
                        eng = nc.vector if ec == 0 else nc.gpsimd
                        eng.tensor_copy(
                            out=out_sb[:, ec * 512 : (ec + 1) * 512], in_=psum_av[:]
                        )
                        if ec == 0:
                            nc.sync.dma_start(
                                num_d[g * P : (g + 1) * P, :512], out_sb[:, :512]
                            )
                            emit_den(g, 8, out_sb)
                    nc.sync.dma_start(
                        num_d[g * P : (g + 1) * P, 512:], out_sb[:, 512:]
                    )

    nc.compile()
    return nc


def _get_nc():
    if "nc" not in _cache:
        _cache["nc"] = _build_nc()
    return _cache["nc"]


def _parity_cols(h):
    return np.concatenate(
        [np.arange(P * (2 * j + h), P * (2 * j + h) + P) for j in range(8)]
    )


def kernel(X, W_q, W_k, W_v, _run_kwargs=None, _results_out=None):
    import ml_dtypes
    from concourse.bass_utils import run_bass_kernel_spmd

    e4 = ml_dtypes.float8_e4m3
    f16 = np.float16
    X = np.asarray(X, dtype=np.float32)
    W_q = np.asarray(W_q, dtype=np.float32)
    W_k = np.asarray(W_k, dtype=np.float32)
    W_v = np.asarray(W_v, dtype=np.float32)
    # scores = (X Wq)(Xk Wk)^T = X wkq^T Xk^T with wkq = Wk Wq^T
    wkq = (W_k @ W_q.T) * np.float32(SW)
    wm_hi = wkq.astype(e4)
    wm_lo = (wkq - wm_hi.astype(np.float32)).astype(e4)
    wvs = W_v * np.float32(WS)
    wv_hi = wvs.astype(e4)
    wv_lo = (wvs - wv_hi.astype(np.float32)).astype(e4)

    xts = [np.ascontiguousarray(X[b].T) for b in range(B)]
    xq8s = [xt.astype(e4) for xt in xts]
    cols = [_parity_cols(0), _parity_cols(1)]
    bands = []
    for h in range(2):
        x = np.arange(256)[None, :]
        p = np.arange(P)[:, None]
        bands.append((x >= p + P * h).astype(f16))

    in_maps = []
    for c in range(N_CORES):
        b, h = divmod(c, 2)
        xk_hi = np.ascontiguousarray(xq8s[b][:, cols[h]])
        xk_lo = (xts[b][:, cols[h]] - xk_hi.astype(np.float32)).astype(e4)
        in_maps.append(
            {
                "xkh": xk_hi,
                "xkl": np.ascontiguousarray(xk_lo),
                "xq": xq8s[b],
                "wmh": wm_hi,
                "wml": wm_lo,
                "wvh": wv_hi,
                "wvl": wv_lo,
                "band": bands[h],
            }
        )

    nc = _get_nc()
    res = None
    for attempt in range(3):
        try:
            res = run_bass_kernel_spmd(
                nc, in_maps, core_ids=list(range(N_CORES)), **(_run_kwargs or {})
            )
            # materialize now: device failures surface lazily at fetch time,
            # and they must land inside this retry loop
            res.results = [
                {k: np.asarray(v) for k, v in r.items()} for r in res.results
            ]
            break
        except Exception:
            # transient device wedges (NRT_EXEC_UNIT_UNRECOVERABLE) usually
            # clear on retry; drop the poisoned PJRT client first
            if attempt == 2:
                raise
            print(f"kernel: device run failed (attempt {attempt}), retrying",
                  file=sys.stderr)
            import time

            try:
                import jax
                import jax.extend.backend

                jax.clear_caches()
                jax.extend.backend.clear_backends()
            except Exception:
                pass
            time.sleep(3)
    if _results_out is not None:
        _results_out.append(res)

    out = np.empty((B, S, D), dtype=np.float32)
    for b in range(B):
        buf = res.results[2 * b]["num"].astype(np.float32) + res.results[
            2 * b + 1
        ]["num"].astype(np.float32)
        out[b] = buf[:, :D] / buf[:, D:]
    return out
